# revision 2
# baseline (speedup 1.0000x reference)
"""BandSplit (gather -> per-band MLP -> scatter-add OLA -> /ola) on 8 TRN2 cores.

Strategy
--------
The whole reference computation is linear in x, so on the host we fold the
per-band pre/post weights, melbank weights, mask, scatter-add and /ola into a
single banded matrix A of shape (C*F, C*F) mapping the (c, f) spectrum of one
(b, t) token to the output spectrum (see _fold_matrix).  The device kernel is
a banded matmul, data-parallel over the 4096 (b, t) tokens across the 8
NeuronCores (512 tokens/core, 4 chunks of 128) with zero cross-core traffic.

v2 layout (vs the v0 baseline):
 * Host pre-transposes x into contraction-major layout (partition = 64
   consecutive f values x 2 input channels), so the PE transposes and the
   gpsimd cast-DMAs are gone entirely and both input channels contract in a
   single matmul pass (halves the number of PE passes; 64-row windows are
   narrower than two 128-row passes: 4764 vs 6956 cols per token chunk).
 * x for low-frequency groups 0..13 ships as fp8 e3m4 (x2 pre-scale folded
   into A) to cut HBM read traffic; high groups stay bf16 where the wide
   bands accumulate too many terms for fp8 (measured rel-err 0.011 vs the
   2e-2 budget).  A stays bf16.
 * All loads are fat HWDGE DMAs on the sync ring; stores go on the scalar
   ring so they never head-of-line block loads.
 * PSUM holds one token-chunk of output (2050 interleaved f32 cols, 5 banks);
   drains are bank-granular, alternating DVE/ACT, so the next chunk's
   matmuls only wait for the one bank they touch.
"""

import numpy as np

_P = 128
_G = 64            # f rows per partition group (x2 channels = 128 partitions)
_C = 2
_F = 1025
_NG = 17           # groups cover f = 0..1087 (1025 real + bias row 1025)
_FP8_GROUPS = 14   # groups 0..13 in e3m4, 14..16 in bf16
_FP8_SCALE = 2.0
_TOK_CORE = 512    # tokens per core
_TCH = 4           # token chunks of 128
_N_CORES = 8


def _fold_matrix(pre_w, pre_b, post_w, post_b, idx, melw, mask, ola_window):
    """Fold the full reference computation into (A, const).

    A: (C, F, C, F) with out[co, fo] = sum_{ci, fi} x[ci, fi] * A[ci, fi, co, fo]
    const: (C, F) additive constant from the biases.
    """
    K, W = idx.shape
    C = _C
    F = ola_window.shape[0]

    pre_w = np.asarray(pre_w, np.float64)
    post_w = np.asarray(post_w, np.float64)
    pre_b = np.asarray(pre_b, np.float64)
    post_b = np.asarray(post_b, np.float64)
    wts = (np.asarray(melw, np.float64) * np.asarray(mask, np.float64))
    msk = np.asarray(mask, np.float64)
    idx = np.asarray(idx)

    M = np.einsum('kio,koj->kij', pre_w, post_w).reshape(K, W, C, W, C)
    vals = M * wts[:, :, None, None, None] * msk[:, None, None, :, None]

    fin = idx[:, :, None, None, None].astype(np.int64)
    fout = idx[:, None, None, :, None].astype(np.int64)
    cin = np.arange(C)[None, None, :, None, None]
    cout = np.arange(C)[None, None, None, None, :]
    flat = ((cin * F + fin) * C + cout) * F + fout
    A = np.bincount(
        np.broadcast_to(flat, vals.shape).ravel(), weights=vals.ravel(),
        minlength=C * F * C * F,
    ).reshape(C, F, C, F)
    A /= ola_window[None, None, None, :]

    bv = (np.einsum('ko,koj->kj', pre_b, post_w) + post_b).reshape(K, W, C)
    bv = bv * msk[:, :, None]
    cflat = (np.arange(C)[None, None, :] * F + idx[:, :, None]).astype(np.int64)
    const = np.bincount(
        np.broadcast_to(cflat, bv.shape).ravel(), weights=bv.ravel(),
        minlength=C * F,
    ).reshape(C, F)
    const /= ola_window[None, :]
    return A, const


def _plan(A, const):
    """Build the banded layout: per-group windows, packed A, segment lists.

    Rows of group j (128 partitions): p = 2*(f - 64j) + ci for f in
    [64j, 64j+64), both channels.  f == 1025 is the bias row (x column == 1).
    Output columns are channel-interleaved: col = 2*fo + co, 2050 total.
    """
    F, C, NG, G = _F, _C, _NG, _G
    # Ap[ci, f, co, fo] over padded f rows (F+1 rows: bias at F)
    Ap = np.zeros((C, NG * G, C, F), np.float64)
    Ap[:, :F] = A
    Ap[0, F] = const

    nzrow = (Ap != 0).any(axis=(0, 2))          # (NG*G, F) over (f, fo)
    wins = []
    for j in range(NG):
        cols = nzrow[j * G:(j + 1) * G].any(axis=0)
        nzc = np.nonzero(cols)[0]
        lo, hi = (int(nzc[0]), int(nzc[-1]) + 1) if len(nzc) else (0, 1)
        wins.append((lo, hi))
    cov = np.zeros(F, bool)
    for lo, hi in wins:
        cov[lo:hi] = True
    assert cov.all(), 'window coverage hole'

    # packed A: [128, TW] with 16-col-aligned per-group blocks
    offs, tw = [], 0
    for j in range(NG):
        offs.append(tw)
        tw += (2 * (wins[j][1] - wins[j][0]) + 15) // 16 * 16
    import ml_dtypes
    ab = np.zeros((_P, tw), ml_dtypes.bfloat16)
    for j in range(NG):
        lo, hi = wins[j]
        blk = Ap[:, j * G:(j + 1) * G, :, lo:hi]       # (ci, 64, co, w)
        blk = blk.transpose(1, 0, 3, 2).reshape(_P, -1)  # p=(f,ci), col=(fo,co)
        if j < _FP8_GROUPS:
            blk = blk / _FP8_SCALE                     # undo x pre-scale
        ab[:, offs[j]:offs[j] + 2 * (hi - lo)] = blk

    # matmul segments per group, split at 512-col PSUM bank boundaries
    segs = []                                          # [(j, s, e)] in order
    for j in range(NG):
        lo2, hi2 = 2 * wins[j][0], 2 * wins[j][1]
        s = lo2
        while s < hi2:
            e = min(hi2, (s // 512 + 1) * 512)
            segs.append((j, s, e))
            s = e
    # first/last toucher of each bank (for start/stop flags), per chunk
    bank_first, bank_last = {}, {}
    for i, (j, s, e) in enumerate(segs):
        b = s // 512
        bank_first.setdefault(b, i)
        bank_last[b] = i
    return wins, offs, tw, ab, segs, bank_first, bank_last


_PROGRAM_CACHE = {}


def _build_program(tw, wins, offs, segs, bank_first, bank_last):
    import concourse.bass as bass  # noqa: F401
    import concourse.tile as tile
    import concourse.mybir as mybir
    from concourse import bacc
    from concourse.masks import make_identity

    f32 = mybir.dt.float32
    bf16 = mybir.dt.bfloat16
    f16 = mybir.dt.float16
    fp8 = mybir.dt.float8e3
    P = _P
    NG, TCH = _NG, _TCH
    N8 = _FP8_GROUPS
    N16 = NG - N8
    W_OUT = 2 * _F                      # 2050 interleaved output cols

    nc = bacc.Bacc("TRN2", target_bir_lowering=False, debug=False,
                   num_devices=_N_CORES)
    xs8 = nc.dram_tensor("xs8", [P, N8 * _TOK_CORE], fp8, kind="ExternalInput")
    xs16 = nc.dram_tensor("xs16", [P, N16 * _TOK_CORE], bf16,
                          kind="ExternalInput")
    ab = nc.dram_tensor("ab", [P, tw], bf16, kind="ExternalInput")
    y = nc.dram_tensor("y", [TCH, P, W_OUT], f16, kind="ExternalOutput")

    # interleave x / A load issue so group j's operands land early together
    X8_SPLITS = [(0, 4), (4, 8), (8, 12), (12, 14)]
    A_SPLITS = [(0, 4), (4, 8), (8, 12), (12, 17)]

    with tile.TileContext(nc) as tc:
        with (
            tc.tile_pool(name="xpool", bufs=1) as xpool,
            tc.tile_pool(name="apool", bufs=1) as apool,
            tc.tile_pool(name="opool", bufs=2) as opool,
            tc.tile_pool(name="idpool", bufs=1) as idpool,
            tc.tile_pool(name="pspool", bufs=1, space="PSUM") as pspool,
        ):
            x8t = xpool.tile([P, N8 * _TOK_CORE], fp8, name="x8")
            x16t = xpool.tile([P, N16 * _TOK_CORE], bf16, name="x16")
            abt = apool.tile([P, tw], bf16, name="abt")
            ident = idpool.tile([P, P], bf16, name="ident")
            make_identity(nc, ident[:])

            S = _TOK_CORE
            for (g0, g1), (a0, a1) in zip(X8_SPLITS, A_SPLITS):
                nc.sync.dma_start(x8t[:, g0 * S:g1 * S], xs8[:, g0 * S:g1 * S])
                o0 = offs[a0]
                o1 = tw if a1 >= NG else offs[a1]
                nc.sync.dma_start(abt[:, o0:o1], ab[:, o0:o1])
            nc.sync.dma_start(x16t[:], xs16[:])

            # PE warmup: trip the HAM clock gate while DMAs land
            warm = pspool.tile([P, P], f32, tag="warm", name="warm")
            for _ in range(24):
                nc.tensor.matmul(warm[:], ident[:], ident[:],
                                 start=True, stop=True)

            def lhsT(j, t):
                if j < N8:
                    return x8t[:, j * S + t * P:j * S + (t + 1) * P]
                return x16t[:, (j - N8) * S + t * P:(j - N8) * S + (t + 1) * P]

            for t in range(TCH):
                pt = pspool.tile([P, W_OUT], f32, tag="pt", name=f"pt_{t}")
                for i, (j, s, e) in enumerate(segs):
                    lo2 = 2 * wins[j][0]
                    o = offs[j]
                    nc.tensor.matmul(
                        pt[:, s:e], lhsT(j, t),
                        abt[:, o + s - lo2:o + e - lo2],
                        start=(bank_first[s // 512] == i),
                        stop=(bank_last[s // 512] == i),
                    )
                # bank-granular drains, alternating engines; stores on the
                # scalar ring so they never block the load stream
                ot = opool.tile([P, W_OUT], f16, tag="out", name=f"out_{t}")
                nc.vector.tensor_copy(ot[:, 0:512], pt[:, 0:512])
                nc.scalar.copy(ot[:, 512:1024], pt[:, 512:1024])
                nc.scalar.dma_start(y[t, :, 0:1024], ot[:, 0:1024])
                nc.vector.tensor_copy(ot[:, 1024:1536], pt[:, 1024:1536])
                nc.scalar.copy(ot[:, 1536:2048], pt[:, 1536:2048])
                nc.vector.tensor_copy(ot[:, 2048:W_OUT], pt[:, 2048:W_OUT])
                nc.scalar.dma_start(y[t, :, 1024:W_OUT], ot[:, 1024:W_OUT])

    nc.compile()
    return nc


def kernel(**inputs):
    import ml_dtypes

    x = np.ascontiguousarray(np.asarray(inputs["x"], np.float32))
    B, C, T, F = x.shape
    assert (B, C, F) == (4, _C, _F), (B, C, F)
    TS = T // _N_CORES

    A, const = _fold_matrix(
        inputs["pre_w"], inputs["pre_b"], inputs["post_w"], inputs["post_b"],
        inputs["idx"], inputs["melw"], inputs["mask"], inputs["ola_window"],
    )
    wins, offs, tw, ab, segs, bank_first, bank_last = _plan(A, const)

    key = (tw, tuple(wins))
    if key not in _PROGRAM_CACHE:
        _PROGRAM_CACHE[key] = _build_program(tw, wins, offs, segs,
                                             bank_first, bank_last)
    nc = _PROGRAM_CACHE[key]

    # host pre-shard: contraction-major x layout per core.
    # xq[ci, f, b, t] with f padded to 1088 (bias row at f=1025 == 1.0)
    NGG = _NG * _G
    xq = np.zeros((_C, NGG, B, T), np.float32)
    xq[:, :F] = x.transpose(1, 3, 0, 2)
    xq[0, F] = 1.0
    # [NG, G, C, B, T] -> partitions p = 2*f_off + ci
    xq = xq.reshape(_C, _NG, _G, B, T).transpose(1, 2, 0, 3, 4)
    x8 = (xq[:_FP8_GROUPS] * _FP8_SCALE).astype(ml_dtypes.float8_e3m4)
    x16 = xq[_FP8_GROUPS:].astype(ml_dtypes.bfloat16)

    in_maps = []
    for m in range(_N_CORES):
        sl8 = x8[:, :, :, :, m * TS:(m + 1) * TS]      # (N8, G, C, B, TS)
        sl16 = x16[:, :, :, :, m * TS:(m + 1) * TS]
        in_maps.append({
            "xs8": np.ascontiguousarray(
                sl8.reshape(_FP8_GROUPS, _P // 2 // 1, _C, _TOK_CORE)
                   .reshape(_FP8_GROUPS, _G * _C, _TOK_CORE)
                   .transpose(1, 0, 2).reshape(_P, -1)),
            "xs16": np.ascontiguousarray(
                sl16.reshape(_NG - _FP8_GROUPS, _G * _C, _TOK_CORE)
                    .transpose(1, 0, 2).reshape(_P, -1)),
            "ab": ab,
        })

    try:
        import antenv.axon_hooks  # noqa: F401
    except ImportError:
        import sys
        import types
        import antenv
        stub = types.ModuleType("antenv.axon_hooks")
        stub.get_axon_ntff_profile_hook = lambda: None
        stub.set_axon_ntff_profile_hook = lambda h: None
        sys.modules["antenv.axon_hooks"] = stub
        antenv.axon_hooks = stub

    from concourse.bass_utils import run_bass_kernel_spmd
    res = run_bass_kernel_spmd(nc, in_maps, core_ids=list(range(_N_CORES)))
    globals()["_LAST_RESULT"] = res

    out = np.empty((B, _C, T, F), np.float32)
    for m in range(_N_CORES):
        ym = res.results[m]["y"].astype(np.float32)     # (TCH, P, 2050)
        ym = ym.reshape(B, TS, F, _C)                   # tok=(b, t_local)
        out[:, :, m * TS:(m + 1) * TS, :] = ym.transpose(0, 3, 1, 2)
    return out


# revision 3
# speedup vs baseline: 1.4990x; 1.4990x over previous
"""BandSplit (gather -> per-band MLP -> scatter-add OLA -> /ola) on 8 TRN2 cores.

Strategy
--------
The whole reference computation is linear in x, so on the host we fold the
per-band pre/post weights, melbank weights, mask, scatter-add and /ola into a
single banded matrix A of shape (C*F, C*F) mapping the (c, f) spectrum of one
(b, t) token to the output spectrum (see _fold_matrix).  The device kernel is
a banded matmul, data-parallel over the 4096 (b, t) tokens across the 8
NeuronCores (512 tokens/core, 4 chunks of 128) with zero cross-core traffic.

v2 layout (vs the v0 baseline):
 * Host pre-transposes x into contraction-major layout (partition = 64
   consecutive f values x 2 input channels), so the PE transposes and the
   gpsimd cast-DMAs are gone entirely and both input channels contract in a
   single matmul pass (halves the number of PE passes; 64-row windows are
   narrower than two 128-row passes: 4764 vs 6956 cols per token chunk).
 * x for low-frequency groups 0..13 ships as fp8 e3m4 (x2 pre-scale folded
   into A) to cut HBM read traffic; high groups stay bf16 where the wide
   bands accumulate too many terms for fp8 (measured rel-err 0.011 vs the
   2e-2 budget).  A stays bf16.
 * All loads are fat HWDGE DMAs on the sync ring; stores go on the scalar
   ring so they never head-of-line block loads.
 * PSUM holds one token-chunk of output (2050 interleaved f32 cols, 5 banks);
   drains are bank-granular, alternating DVE/ACT, so the next chunk's
   matmuls only wait for the one bank they touch.
"""

import numpy as np

_P = 128
_G = 64            # f rows per partition group (x2 channels = 128 partitions)
_C = 2
_F = 1025
_NG = 17           # groups cover f = 0..1087 (1025 real + bias row 1025)
_FP8_GROUPS = 14   # groups 0..13 in e3m4, 14..16 in bf16
_FP8_SCALE = 2.0
_TOK_CORE = 512    # tokens per core
_TCH = 4           # token chunks of 128
_N_CORES = 8


def _fold_matrix(pre_w, pre_b, post_w, post_b, idx, melw, mask, ola_window):
    """Fold the full reference computation into (A, const).

    A: (C, F, C, F) with out[co, fo] = sum_{ci, fi} x[ci, fi] * A[ci, fi, co, fo]
    const: (C, F) additive constant from the biases.
    """
    K, W = idx.shape
    C = _C
    F = ola_window.shape[0]

    pre_w = np.asarray(pre_w, np.float64)
    post_w = np.asarray(post_w, np.float64)
    pre_b = np.asarray(pre_b, np.float64)
    post_b = np.asarray(post_b, np.float64)
    wts = (np.asarray(melw, np.float64) * np.asarray(mask, np.float64))
    msk = np.asarray(mask, np.float64)
    idx = np.asarray(idx)

    M = np.einsum('kio,koj->kij', pre_w, post_w).reshape(K, W, C, W, C)
    vals = M * wts[:, :, None, None, None] * msk[:, None, None, :, None]

    fin = idx[:, :, None, None, None].astype(np.int64)
    fout = idx[:, None, None, :, None].astype(np.int64)
    cin = np.arange(C)[None, None, :, None, None]
    cout = np.arange(C)[None, None, None, None, :]
    flat = ((cin * F + fin) * C + cout) * F + fout
    A = np.bincount(
        np.broadcast_to(flat, vals.shape).ravel(), weights=vals.ravel(),
        minlength=C * F * C * F,
    ).reshape(C, F, C, F)
    A /= ola_window[None, None, None, :]

    bv = (np.einsum('ko,koj->kj', pre_b, post_w) + post_b).reshape(K, W, C)
    bv = bv * msk[:, :, None]
    cflat = (np.arange(C)[None, None, :] * F + idx[:, :, None]).astype(np.int64)
    const = np.bincount(
        np.broadcast_to(cflat, bv.shape).ravel(), weights=bv.ravel(),
        minlength=C * F,
    ).reshape(C, F)
    const /= ola_window[None, :]
    return A, const


def _plan(A, const):
    """Build the banded layout: per-group windows, packed A, segment lists.

    Rows of group j (128 partitions): p = 2*(f - 64j) + ci for f in
    [64j, 64j+64), both channels.  f == 1025 is the bias row (x column == 1).
    Output columns are channel-interleaved: col = 2*fo + co, 2050 total.
    """
    F, C, NG, G = _F, _C, _NG, _G
    # Ap[ci, f, co, fo] over padded f rows (F+1 rows: bias at F)
    Ap = np.zeros((C, NG * G, C, F), np.float64)
    Ap[:, :F] = A
    Ap[0, F] = const

    nzrow = (Ap != 0).any(axis=(0, 2))          # (NG*G, F) over (f, fo)
    wins = []
    for j in range(NG):
        cols = nzrow[j * G:(j + 1) * G].any(axis=0)
        nzc = np.nonzero(cols)[0]
        lo, hi = (int(nzc[0]), int(nzc[-1]) + 1) if len(nzc) else (0, 1)
        wins.append((lo, hi))
    cov = np.zeros(F, bool)
    for lo, hi in wins:
        cov[lo:hi] = True
    assert cov.all(), 'window coverage hole'

    # packed A: [128, TW] with 16-col-aligned per-group blocks
    offs, tw = [], 0
    for j in range(NG):
        offs.append(tw)
        tw += (2 * (wins[j][1] - wins[j][0]) + 15) // 16 * 16
    import ml_dtypes
    ab = np.zeros((_P, tw), ml_dtypes.bfloat16)
    for j in range(NG):
        lo, hi = wins[j]
        blk = Ap[:, j * G:(j + 1) * G, :, lo:hi]       # (ci, 64, co, w)
        blk = blk.transpose(1, 0, 3, 2).reshape(_P, -1)  # p=(f,ci), col=(fo,co)
        if j < _FP8_GROUPS:
            blk = blk / _FP8_SCALE                     # undo x pre-scale
        ab[:, offs[j]:offs[j] + 2 * (hi - lo)] = blk

    # matmul segments per group, split at 512-col PSUM bank boundaries
    segs = []                                          # [(j, s, e)] in order
    for j in range(NG):
        lo2, hi2 = 2 * wins[j][0], 2 * wins[j][1]
        s = lo2
        while s < hi2:
            e = min(hi2, (s // 512 + 1) * 512)
            segs.append((j, s, e))
            s = e
    # first/last toucher of each bank (for start/stop flags), per chunk
    bank_first, bank_last = {}, {}
    for i, (j, s, e) in enumerate(segs):
        b = s // 512
        bank_first.setdefault(b, i)
        bank_last[b] = i
    return wins, offs, tw, ab, segs, bank_first, bank_last


_PROGRAM_CACHE = {}


def _build_program(tw, wins, offs, segs, bank_first, bank_last):
    import concourse.bass as bass  # noqa: F401
    import concourse.tile as tile
    import concourse.mybir as mybir
    from concourse import bacc
    from concourse.masks import make_identity

    f32 = mybir.dt.float32
    bf16 = mybir.dt.bfloat16
    f16 = mybir.dt.float16
    fp8 = mybir.dt.float8e3
    P = _P
    NG, TCH = _NG, _TCH
    N8 = _FP8_GROUPS
    N16 = NG - N8
    W_OUT = 2 * _F                      # 2050 interleaved output cols

    nc = bacc.Bacc("TRN2", target_bir_lowering=False, debug=False,
                   num_devices=_N_CORES)
    xs8 = nc.dram_tensor("xs8", [P, N8 * _TOK_CORE], fp8, kind="ExternalInput")
    xs16 = nc.dram_tensor("xs16", [P, N16 * _TOK_CORE], bf16,
                          kind="ExternalInput")
    ab = nc.dram_tensor("ab", [P, tw], bf16, kind="ExternalInput")
    y = nc.dram_tensor("y", [TCH, P, W_OUT], f16, kind="ExternalOutput")

    # loads go out on three DMA rings in parallel (sync: x fp8, scalar: A,
    # gpsimd: x bf16); stores ride the sync ring after its loads are issued
    X8_SPLITS = [(0, 5), (5, 10), (10, 14)]
    A_SPLITS = [(0, 6), (6, 11), (11, 17)]
    NBANK = (W_OUT + 511) // 512            # 5 PSUM banks (last holds 2 cols)

    with tile.TileContext(nc) as tc:
        with (
            tc.tile_pool(name="xpool", bufs=1) as xpool,
            tc.tile_pool(name="apool", bufs=1) as apool,
            tc.tile_pool(name="opool", bufs=2) as opool,
            tc.tile_pool(name="idpool", bufs=1) as idpool,
            tc.tile_pool(name="pspool", bufs=1, space="PSUM") as pspool,
        ):
            x8t = xpool.tile([P, N8 * _TOK_CORE], fp8, name="x8")
            x16t = xpool.tile([P, N16 * _TOK_CORE], bf16, name="x16")
            abt = apool.tile([P, tw], bf16, name="abt")
            ident = idpool.tile([P, P], bf16, name="ident")
            make_identity(nc, ident[:])

            S = _TOK_CORE
            for g0, g1 in X8_SPLITS:
                nc.sync.dma_start(x8t[:, g0 * S:g1 * S], xs8[:, g0 * S:g1 * S])
            for a0, a1 in A_SPLITS:
                o0 = offs[a0]
                o1 = tw if a1 >= NG else offs[a1]
                nc.scalar.dma_start(abt[:, o0:o1], ab[:, o0:o1])
            nc.gpsimd.dma_start(x16t[:], xs16[:])

            # PE warmup: trip the HAM clock gate while DMAs land
            warm = pspool.tile([P, P], f32, tag="warm", name="warm")
            for _ in range(20):
                nc.tensor.matmul(warm[:], ident[:], ident[:],
                                 start=True, stop=True)

            def lhsT(j, t):
                if j < N8:
                    return x8t[:, j * S + t * P:j * S + (t + 1) * P]
                return x16t[:, (j - N8) * S + t * P:(j - N8) * S + (t + 1) * P]

            # one PSUM tile per 512-col bank so drain dependencies are
            # bank-granular: bank b of chunk t drains as soon as its last
            # accumulating matmul retires, overlapping the rest of the chunk
            # and unblocking chunk t+1's early matmuls
            def bank_tile(t, b):
                w = min(512, W_OUT - b * 512)
                return pspool.tile([P, w], f32, tag=f"ptb{b}",
                                   name=f"pt_{t}_{b}")

            for t in range(TCH):
                pts = [bank_tile(t, b) for b in range(NBANK)]
                for i, (j, s, e) in enumerate(segs):
                    b = s // 512
                    lo2 = 2 * wins[j][0]
                    o = offs[j]
                    nc.tensor.matmul(
                        pts[b][:, s - b * 512:e - b * 512], lhsT(j, t),
                        abt[:, o + s - lo2:o + e - lo2],
                        start=(bank_first[b] == i),
                        stop=(bank_last[b] == i),
                    )
                ot = opool.tile([P, W_OUT], f16, tag="out", name=f"out_{t}")
                nc.vector.tensor_copy(ot[:, 0:512], pts[0][:])
                nc.scalar.copy(ot[:, 512:1024], pts[1][:])
                nc.sync.dma_start(y[t, :, 0:1024], ot[:, 0:1024])
                nc.vector.tensor_copy(ot[:, 1024:1536], pts[2][:])
                nc.scalar.copy(ot[:, 1536:2048], pts[3][:])
                nc.vector.tensor_copy(ot[:, 2048:W_OUT], pts[4][:])
                nc.sync.dma_start(y[t, :, 1024:W_OUT], ot[:, 1024:W_OUT])

    nc.compile()
    return nc


def kernel(**inputs):
    import ml_dtypes

    x = np.ascontiguousarray(np.asarray(inputs["x"], np.float32))
    B, C, T, F = x.shape
    assert (B, C, F) == (4, _C, _F), (B, C, F)
    TS = T // _N_CORES

    A, const = _fold_matrix(
        inputs["pre_w"], inputs["pre_b"], inputs["post_w"], inputs["post_b"],
        inputs["idx"], inputs["melw"], inputs["mask"], inputs["ola_window"],
    )
    wins, offs, tw, ab, segs, bank_first, bank_last = _plan(A, const)

    key = (tw, tuple(wins))
    if key not in _PROGRAM_CACHE:
        _PROGRAM_CACHE[key] = _build_program(tw, wins, offs, segs,
                                             bank_first, bank_last)
    nc = _PROGRAM_CACHE[key]

    # host pre-shard: contraction-major x layout per core.
    # xq[ci, f, b, t] with f padded to 1088 (bias row at f=1025 == 1.0)
    NGG = _NG * _G
    xq = np.zeros((_C, NGG, B, T), np.float32)
    xq[:, :F] = x.transpose(1, 3, 0, 2)
    xq[0, F] = 1.0
    # [NG, G, C, B, T] -> partitions p = 2*f_off + ci
    xq = xq.reshape(_C, _NG, _G, B, T).transpose(1, 2, 0, 3, 4)
    x8 = (xq[:_FP8_GROUPS] * _FP8_SCALE).astype(ml_dtypes.float8_e3m4)
    x16 = xq[_FP8_GROUPS:].astype(ml_dtypes.bfloat16)

    in_maps = []
    for m in range(_N_CORES):
        sl8 = x8[:, :, :, :, m * TS:(m + 1) * TS]      # (N8, G, C, B, TS)
        sl16 = x16[:, :, :, :, m * TS:(m + 1) * TS]
        in_maps.append({
            "xs8": np.ascontiguousarray(
                sl8.reshape(_FP8_GROUPS, _P // 2 // 1, _C, _TOK_CORE)
                   .reshape(_FP8_GROUPS, _G * _C, _TOK_CORE)
                   .transpose(1, 0, 2).reshape(_P, -1)),
            "xs16": np.ascontiguousarray(
                sl16.reshape(_NG - _FP8_GROUPS, _G * _C, _TOK_CORE)
                    .transpose(1, 0, 2).reshape(_P, -1)),
            "ab": ab,
        })

    try:
        import antenv.axon_hooks  # noqa: F401
    except ImportError:
        import sys
        import types
        import antenv
        stub = types.ModuleType("antenv.axon_hooks")
        stub.get_axon_ntff_profile_hook = lambda: None
        stub.set_axon_ntff_profile_hook = lambda h: None
        sys.modules["antenv.axon_hooks"] = stub
        antenv.axon_hooks = stub

    from concourse.bass_utils import run_bass_kernel_spmd
    res = run_bass_kernel_spmd(nc, in_maps, core_ids=list(range(_N_CORES)))
    globals()["_LAST_RESULT"] = res

    out = np.empty((B, _C, T, F), np.float32)
    for m in range(_N_CORES):
        ym = res.results[m]["y"].astype(np.float32)     # (TCH, P, 2050)
        ym = ym.reshape(B, TS, F, _C)                   # tok=(b, t_local)
        out[:, :, m * TS:(m + 1) * TS, :] = ym.transpose(0, 3, 1, 2)
    return out


# revision 6
# speedup vs baseline: 1.5073x; 1.0055x over previous
"""BandSplit (gather -> per-band MLP -> scatter-add OLA -> /ola) on 8 TRN2 cores.

Strategy
--------
The whole reference computation is linear in x, so on the host we fold the
per-band pre/post weights, melbank weights, mask, scatter-add and /ola into a
single banded matrix A of shape (C*F, C*F) mapping the (c, f) spectrum of one
(b, t) token to the output spectrum (see _fold_matrix).  The device kernel is
a banded matmul, data-parallel over the 4096 (b, t) tokens across the 8
NeuronCores (512 tokens/core, 4 chunks of 128) with zero cross-core traffic.

v2 layout (vs the v0 baseline):
 * Host pre-transposes x into contraction-major layout (partition = 64
   consecutive f values x 2 input channels), so the PE transposes and the
   gpsimd cast-DMAs are gone entirely and both input channels contract in a
   single matmul pass (halves the number of PE passes; 64-row windows are
   narrower than two 128-row passes: 4764 vs 6956 cols per token chunk).
 * x for low-frequency groups 0..13 ships as fp8 e3m4 (x2 pre-scale folded
   into A) to cut HBM read traffic; high groups stay bf16 where the wide
   bands accumulate too many terms for fp8 (measured rel-err 0.011 vs the
   2e-2 budget).  A stays bf16.
 * All loads are fat HWDGE DMAs on the sync ring; stores go on the scalar
   ring so they never head-of-line block loads.
 * PSUM holds one token-chunk of output (2050 interleaved f32 cols, 5 banks);
   drains are bank-granular, alternating DVE/ACT, so the next chunk's
   matmuls only wait for the one bank they touch.
"""

import numpy as np

_P = 128
_G = 64            # f rows per partition group (x2 channels = 128 partitions)
_C = 2
_F = 1025
_NG = 17           # groups cover f = 0..1087 (1025 real + bias row 1025)
_FP8_GROUPS = 14   # groups 0..13 in e3m4, 14..16 in bf16
_FP8_SCALE = 2.0
_TOK_CORE = 512    # tokens per core
_TCH = 4           # token chunks of 128
_N_CORES = 8


def _fold_matrix(pre_w, pre_b, post_w, post_b, idx, melw, mask, ola_window):
    """Fold the full reference computation into (A, const).

    A: (C, F, C, F) with out[co, fo] = sum_{ci, fi} x[ci, fi] * A[ci, fi, co, fo]
    const: (C, F) additive constant from the biases.
    """
    K, W = idx.shape
    C = _C
    F = ola_window.shape[0]

    pre_w = np.asarray(pre_w, np.float64)
    post_w = np.asarray(post_w, np.float64)
    pre_b = np.asarray(pre_b, np.float64)
    post_b = np.asarray(post_b, np.float64)
    wts = (np.asarray(melw, np.float64) * np.asarray(mask, np.float64))
    msk = np.asarray(mask, np.float64)
    idx = np.asarray(idx)

    M = np.einsum('kio,koj->kij', pre_w, post_w).reshape(K, W, C, W, C)
    vals = M * wts[:, :, None, None, None] * msk[:, None, None, :, None]

    fin = idx[:, :, None, None, None].astype(np.int64)
    fout = idx[:, None, None, :, None].astype(np.int64)
    cin = np.arange(C)[None, None, :, None, None]
    cout = np.arange(C)[None, None, None, None, :]
    flat = ((cin * F + fin) * C + cout) * F + fout
    A = np.bincount(
        np.broadcast_to(flat, vals.shape).ravel(), weights=vals.ravel(),
        minlength=C * F * C * F,
    ).reshape(C, F, C, F)
    A /= ola_window[None, None, None, :]

    bv = (np.einsum('ko,koj->kj', pre_b, post_w) + post_b).reshape(K, W, C)
    bv = bv * msk[:, :, None]
    cflat = (np.arange(C)[None, None, :] * F + idx[:, :, None]).astype(np.int64)
    const = np.bincount(
        np.broadcast_to(cflat, bv.shape).ravel(), weights=bv.ravel(),
        minlength=C * F,
    ).reshape(C, F)
    const /= ola_window[None, :]
    return A, const


def _plan(A, const):
    """Build the banded layout: per-group windows, packed A, segment lists.

    Rows of group j (128 partitions): p = 2*(f - 64j) + ci for f in
    [64j, 64j+64), both channels.  f == 1025 is the bias row (x column == 1).
    Output columns are channel-interleaved: col = 2*fo + co, 2050 total.
    """
    F, C, NG, G = _F, _C, _NG, _G
    # Ap[ci, f, co, fo] over padded f rows (F+1 rows: bias at F)
    Ap = np.zeros((C, NG * G, C, F), np.float64)
    Ap[:, :F] = A
    Ap[0, F] = const

    nzrow = (Ap != 0).any(axis=(0, 2))          # (NG*G, F) over (f, fo)
    wins = []
    for j in range(NG):
        cols = nzrow[j * G:(j + 1) * G].any(axis=0)
        nzc = np.nonzero(cols)[0]
        lo, hi = (int(nzc[0]), int(nzc[-1]) + 1) if len(nzc) else (0, 1)
        wins.append((lo, hi))
    cov = np.zeros(F, bool)
    for lo, hi in wins:
        cov[lo:hi] = True
    assert cov.all(), 'window coverage hole'

    # packed A: [128, TW] with 16-col-aligned per-group blocks
    offs, tw = [], 0
    for j in range(NG):
        offs.append(tw)
        tw += (2 * (wins[j][1] - wins[j][0]) + 15) // 16 * 16
    import ml_dtypes
    ab = np.zeros((_P, tw), ml_dtypes.bfloat16)
    for j in range(NG):
        lo, hi = wins[j]
        blk = Ap[:, j * G:(j + 1) * G, :, lo:hi]       # (ci, 64, co, w)
        blk = blk.transpose(1, 0, 3, 2).reshape(_P, -1)  # p=(f,ci), col=(fo,co)
        if j < _FP8_GROUPS:
            blk = blk / _FP8_SCALE                     # undo x pre-scale
        ab[:, offs[j]:offs[j] + 2 * (hi - lo)] = blk

    # matmul segments per group, split at 512-col PSUM bank boundaries
    segs = []                                          # [(j, s, e)] in order
    for j in range(NG):
        lo2, hi2 = 2 * wins[j][0], 2 * wins[j][1]
        s = lo2
        while s < hi2:
            e = min(hi2, (s // 512 + 1) * 512)
            segs.append((j, s, e))
            s = e
    # first/last toucher of each bank (for start/stop flags), per chunk
    bank_first, bank_last = {}, {}
    for i, (j, s, e) in enumerate(segs):
        b = s // 512
        bank_first.setdefault(b, i)
        bank_last[b] = i
    return wins, offs, tw, ab, segs, bank_first, bank_last


_PROGRAM_CACHE = {}


def _build_program(tw, wins, offs, segs, bank_first, bank_last):
    import concourse.bass as bass  # noqa: F401
    import concourse.tile as tile
    import concourse.mybir as mybir
    from concourse import bacc
    from concourse.masks import make_identity

    f32 = mybir.dt.float32
    bf16 = mybir.dt.bfloat16
    f16 = mybir.dt.float16
    fp8 = mybir.dt.float8e3
    P = _P
    NG, TCH = _NG, _TCH
    N8 = _FP8_GROUPS
    N16 = NG - N8
    W_OUT = 2 * _F                      # 2050 interleaved output cols

    nc = bacc.Bacc("TRN2", target_bir_lowering=False, debug=False,
                   num_devices=_N_CORES)
    xs8 = nc.dram_tensor("xs8", [P, N8 * _TOK_CORE], fp8, kind="ExternalInput")
    xs16 = nc.dram_tensor("xs16", [P, N16 * _TOK_CORE], bf16,
                          kind="ExternalInput")
    ab = nc.dram_tensor("ab", [P, tw], bf16, kind="ExternalInput")
    y = nc.dram_tensor("y", [TCH, P, W_OUT], f16, kind="ExternalOutput")

    # loads stream on three DMA rings concurrently (sync: x fp8, scalar: A,
    # gpsimd: x bf16); stores alternate between the sync and gpsimd rings
    X8_SPLITS = [(0, 2), (2, 5), (5, 8), (8, 11), (11, 14)]
    A_SPLITS = [(0, 3), (3, 6), (6, 9), (9, 12), (12, 17)]
    NBANK = (W_OUT + 511) // 512            # 5 PSUM banks (last holds 2 cols)

    with tile.TileContext(nc) as tc:
        with (
            tc.tile_pool(name="xpool", bufs=1) as xpool,
            tc.tile_pool(name="apool", bufs=1) as apool,
            tc.tile_pool(name="opool", bufs=2) as opool,
            tc.tile_pool(name="idpool", bufs=1) as idpool,
            tc.tile_pool(name="pspool", bufs=1, space="PSUM") as pspool,
        ):
            x8t = xpool.tile([P, N8 * _TOK_CORE], fp8, name="x8")
            x16t = xpool.tile([P, N16 * _TOK_CORE], bf16, name="x16")
            abt = apool.tile([P, tw], bf16, name="abt")
            ident = idpool.tile([P, P], bf16, name="ident")
            make_identity(nc, ident[:])

            S = _TOK_CORE
            for g0, g1 in X8_SPLITS:
                nc.sync.dma_start(x8t[:, g0 * S:g1 * S], xs8[:, g0 * S:g1 * S])
            for a0, a1 in A_SPLITS:
                o0 = offs[a0]
                o1 = tw if a1 >= NG else offs[a1]
                nc.scalar.dma_start(abt[:, o0:o1], ab[:, o0:o1])
            nc.gpsimd.dma_start(x16t[:], xs16[:])

            # PE warmup: >=3.4us of continuous matmuls trips the HAM clock
            # gate to 2.4 GHz while the DMAs land
            warm = pspool.tile([P, P], f32, tag="warm", name="warm")
            for _ in range(36):
                nc.tensor.matmul(warm[:], ident[:], ident[:],
                                 start=True, stop=True)

            def lhsT(j, t):
                if j < N8:
                    return x8t[:, j * S + t * P:j * S + (t + 1) * P]
                return x16t[:, (j - N8) * S + t * P:(j - N8) * S + (t + 1) * P]

            # one PSUM tile per 512-col bank so drain dependencies are
            # bank-granular: bank b of chunk t drains as soon as its last
            # accumulating matmul retires, overlapping the rest of the chunk
            # and unblocking chunk t+1's early matmuls
            def bank_tile(t, b):
                w = min(512, W_OUT - b * 512)
                return pspool.tile([P, w], f32, tag=f"ptb{b}",
                                   name=f"pt_{t}_{b}")

            for t in range(TCH):
                pts = [bank_tile(t, b) for b in range(NBANK)]
                for i, (j, s, e) in enumerate(segs):
                    b = s // 512
                    lo2 = 2 * wins[j][0]
                    o = offs[j]
                    nc.tensor.matmul(
                        pts[b][:, s - b * 512:e - b * 512], lhsT(j, t),
                        abt[:, o + s - lo2:o + e - lo2],
                        start=(bank_first[b] == i),
                        stop=(bank_last[b] == i),
                    )
                ot = opool.tile([P, W_OUT], f16, tag="out", name=f"out_{t}")
                nc.vector.tensor_copy(ot[:, 0:512], pts[0][:])
                nc.scalar.copy(ot[:, 512:1024], pts[1][:])
                nc.sync.dma_start(y[t, :, 0:1024], ot[:, 0:1024])
                nc.vector.tensor_copy(ot[:, 1024:1536], pts[2][:])
                nc.scalar.copy(ot[:, 1536:2048], pts[3][:])
                nc.vector.tensor_copy(ot[:, 2048:W_OUT], pts[4][:])
                nc.gpsimd.dma_start(y[t, :, 1024:W_OUT], ot[:, 1024:W_OUT])

    nc.compile()
    return nc


def kernel(**inputs):
    import ml_dtypes

    x = np.ascontiguousarray(np.asarray(inputs["x"], np.float32))
    B, C, T, F = x.shape
    assert (B, C, F) == (4, _C, _F), (B, C, F)
    TS = T // _N_CORES

    A, const = _fold_matrix(
        inputs["pre_w"], inputs["pre_b"], inputs["post_w"], inputs["post_b"],
        inputs["idx"], inputs["melw"], inputs["mask"], inputs["ola_window"],
    )
    wins, offs, tw, ab, segs, bank_first, bank_last = _plan(A, const)

    key = (tw, tuple(wins))
    if key not in _PROGRAM_CACHE:
        _PROGRAM_CACHE[key] = _build_program(tw, wins, offs, segs,
                                             bank_first, bank_last)
    nc = _PROGRAM_CACHE[key]

    # host pre-shard: contraction-major x layout per core.
    # xq[ci, f, b, t] with f padded to 1088 (bias row at f=1025 == 1.0)
    NGG = _NG * _G
    xq = np.zeros((_C, NGG, B, T), np.float32)
    xq[:, :F] = x.transpose(1, 3, 0, 2)
    xq[0, F] = 1.0
    # [NG, G, C, B, T] -> partitions p = 2*f_off + ci
    xq = xq.reshape(_C, _NG, _G, B, T).transpose(1, 2, 0, 3, 4)
    x8 = (xq[:_FP8_GROUPS] * _FP8_SCALE).astype(ml_dtypes.float8_e3m4)
    x16 = xq[_FP8_GROUPS:].astype(ml_dtypes.bfloat16)

    in_maps = []
    for m in range(_N_CORES):
        sl8 = x8[:, :, :, :, m * TS:(m + 1) * TS]      # (N8, G, C, B, TS)
        sl16 = x16[:, :, :, :, m * TS:(m + 1) * TS]
        in_maps.append({
            "xs8": np.ascontiguousarray(
                sl8.reshape(_FP8_GROUPS, _P // 2 // 1, _C, _TOK_CORE)
                   .reshape(_FP8_GROUPS, _G * _C, _TOK_CORE)
                   .transpose(1, 0, 2).reshape(_P, -1)),
            "xs16": np.ascontiguousarray(
                sl16.reshape(_NG - _FP8_GROUPS, _G * _C, _TOK_CORE)
                    .transpose(1, 0, 2).reshape(_P, -1)),
            "ab": ab,
        })

    try:
        import antenv.axon_hooks  # noqa: F401
    except ImportError:
        import sys
        import types
        import antenv
        stub = types.ModuleType("antenv.axon_hooks")
        stub.get_axon_ntff_profile_hook = lambda: None
        stub.set_axon_ntff_profile_hook = lambda h: None
        sys.modules["antenv.axon_hooks"] = stub
        antenv.axon_hooks = stub

    from concourse.bass_utils import run_bass_kernel_spmd
    res = run_bass_kernel_spmd(nc, in_maps, core_ids=list(range(_N_CORES)))
    globals()["_LAST_RESULT"] = res

    out = np.empty((B, _C, T, F), np.float32)
    for m in range(_N_CORES):
        ym = res.results[m]["y"].astype(np.float32)     # (TCH, P, 2050)
        ym = ym.reshape(B, TS, F, _C)                   # tok=(b, t_local)
        out[:, :, m * TS:(m + 1) * TS, :] = ym.transpose(0, 3, 1, 2)
    return out


# revision 16
# speedup vs baseline: 1.5732x; 1.0437x over previous
"""BandSplit (gather -> per-band MLP -> scatter-add OLA -> /ola) on 8 TRN2 cores.

Strategy
--------
The whole reference computation is linear in x, so on the host we fold the
per-band pre/post weights, melbank weights, mask, scatter-add and /ola into a
single banded matrix A of shape (C*F, C*F) mapping the (c, f) spectrum of one
(b, t) token to the output spectrum (see _fold_matrix).  The device kernel is
a banded matmul, data-parallel over the 4096 (b, t) tokens across the 8
NeuronCores (512 tokens/core, 4 chunks of 128) with zero cross-core traffic.

v2 layout (vs the v0 baseline):
 * Host pre-transposes x into contraction-major layout (partition = 64
   consecutive f values x 2 input channels), so the PE transposes and the
   gpsimd cast-DMAs are gone entirely and both input channels contract in a
   single matmul pass (halves the number of PE passes; 64-row windows are
   narrower than two 128-row passes: 4764 vs 6956 cols per token chunk).
 * x for low-frequency groups 0..13 ships as fp8 e3m4 (x2 pre-scale folded
   into A) to cut HBM read traffic; high groups stay bf16 where the wide
   bands accumulate too many terms for fp8 (measured rel-err 0.011 vs the
   2e-2 budget).  A stays bf16.
 * All loads are fat HWDGE DMAs on the sync ring; stores go on the scalar
   ring so they never head-of-line block loads.
 * PSUM holds one token-chunk of output (2050 interleaved f32 cols, 5 banks);
   drains are bank-granular, alternating DVE/ACT, so the next chunk's
   matmuls only wait for the one bank they touch.
"""

import numpy as np

_P = 128
_G = 64            # f rows per partition group (x2 channels = 128 partitions)
_C = 2
_F = 1025
_NG = 17           # groups cover f = 0..1087 (1025 real + bias row 1025)
_FP8_GROUPS = 14   # groups 0..13 in e3m4, 14..16 in bf16
_FP8_SCALE = 2.0
_TOK_CORE = 512    # tokens per core
_TCH = 4           # token chunks of 128
_N_CORES = 8


def _fold_matrix(pre_w, pre_b, post_w, post_b, idx, melw, mask, ola_window):
    """Fold the full reference computation into (A, const).

    A: (C, F, C, F) with out[co, fo] = sum_{ci, fi} x[ci, fi] * A[ci, fi, co, fo]
    const: (C, F) additive constant from the biases.
    """
    K, W = idx.shape
    C = _C
    F = ola_window.shape[0]

    pre_w = np.asarray(pre_w, np.float64)
    post_w = np.asarray(post_w, np.float64)
    pre_b = np.asarray(pre_b, np.float64)
    post_b = np.asarray(post_b, np.float64)
    wts = (np.asarray(melw, np.float64) * np.asarray(mask, np.float64))
    msk = np.asarray(mask, np.float64)
    idx = np.asarray(idx)

    M = np.einsum('kio,koj->kij', pre_w, post_w).reshape(K, W, C, W, C)
    vals = M * wts[:, :, None, None, None] * msk[:, None, None, :, None]

    fin = idx[:, :, None, None, None].astype(np.int64)
    fout = idx[:, None, None, :, None].astype(np.int64)
    cin = np.arange(C)[None, None, :, None, None]
    cout = np.arange(C)[None, None, None, None, :]
    flat = ((cin * F + fin) * C + cout) * F + fout
    A = np.bincount(
        np.broadcast_to(flat, vals.shape).ravel(), weights=vals.ravel(),
        minlength=C * F * C * F,
    ).reshape(C, F, C, F)
    A /= ola_window[None, None, None, :]

    bv = (np.einsum('ko,koj->kj', pre_b, post_w) + post_b).reshape(K, W, C)
    bv = bv * msk[:, :, None]
    cflat = (np.arange(C)[None, None, :] * F + idx[:, :, None]).astype(np.int64)
    const = np.bincount(
        np.broadcast_to(cflat, bv.shape).ravel(), weights=bv.ravel(),
        minlength=C * F,
    ).reshape(C, F)
    const /= ola_window[None, :]
    return A, const


def _plan(A, const):
    """Build the banded layout: per-group windows, packed A, segment lists.

    Rows of group j (128 partitions): p = 2*(f - 64j) + ci for f in
    [64j, 64j+64), both channels.  f == 1025 is the bias row (x column == 1).
    Output columns are channel-interleaved: col = 2*fo + co, 2050 total.
    """
    F, C, NG, G = _F, _C, _NG, _G
    # Ap[ci, f, co, fo] over padded f rows (F+1 rows: bias at F)
    Ap = np.zeros((C, NG * G, C, F), np.float64)
    Ap[:, :F] = A
    Ap[0, F] = const

    nzrow = (Ap != 0).any(axis=(0, 2))          # (NG*G, F) over (f, fo)
    wins = []
    for j in range(NG):
        cols = nzrow[j * G:(j + 1) * G].any(axis=0)
        nzc = np.nonzero(cols)[0]
        lo, hi = (int(nzc[0]), int(nzc[-1]) + 1) if len(nzc) else (0, 1)
        wins.append((lo, hi))
    cov = np.zeros(F, bool)
    for lo, hi in wins:
        cov[lo:hi] = True
    assert cov.all(), 'window coverage hole'

    # packed A: [128, TW] with 16-col-aligned per-group blocks
    offs, tw = [], 0
    for j in range(NG):
        offs.append(tw)
        tw += (2 * (wins[j][1] - wins[j][0]) + 15) // 16 * 16
    import ml_dtypes
    ab = np.zeros((_P, tw), ml_dtypes.bfloat16)
    for j in range(NG):
        lo, hi = wins[j]
        blk = Ap[:, j * G:(j + 1) * G, :, lo:hi]       # (ci, 64, co, w)
        blk = blk.transpose(1, 0, 3, 2).reshape(_P, -1)  # p=(f,ci), col=(fo,co)
        if j < _FP8_GROUPS:
            blk = blk / _FP8_SCALE                     # undo x pre-scale
        ab[:, offs[j]:offs[j] + 2 * (hi - lo)] = blk

    # matmul segments per group, split at 512-col PSUM bank boundaries
    segs = []                                          # [(j, s, e)] in order
    for j in range(NG):
        lo2, hi2 = 2 * wins[j][0], 2 * wins[j][1]
        s = lo2
        while s < hi2:
            e = min(hi2, (s // 512 + 1) * 512)
            segs.append((j, s, e))
            s = e
    # first/last toucher of each bank (for start/stop flags), per chunk
    bank_first, bank_last = {}, {}
    for i, (j, s, e) in enumerate(segs):
        b = s // 512
        bank_first.setdefault(b, i)
        bank_last[b] = i
    return wins, offs, tw, ab, segs, bank_first, bank_last


_PROGRAM_CACHE = {}


def _build_program(tw, wins, offs, segs, bank_first, bank_last):
    import concourse.bass as bass  # noqa: F401
    import concourse.tile as tile
    import concourse.mybir as mybir
    from concourse import bacc
    from concourse.masks import make_identity

    f32 = mybir.dt.float32
    bf16 = mybir.dt.bfloat16
    f16 = mybir.dt.float16
    fp8 = mybir.dt.float8e3
    P = _P
    NG, TCH = _NG, _TCH
    N8 = _FP8_GROUPS
    N16 = NG - N8
    W_OUT = 2 * _F                      # 2050 interleaved output cols

    nc = bacc.Bacc("TRN2", target_bir_lowering=False, debug=False,
                   num_devices=_N_CORES)
    xs8 = nc.dram_tensor("xs8", [P, N8 * _TOK_CORE], fp8, kind="ExternalInput")
    xs16 = nc.dram_tensor("xs16", [P, N16 * _TOK_CORE], bf16,
                          kind="ExternalInput")
    ab = nc.dram_tensor("ab", [P, tw], bf16, kind="ExternalInput")
    y = nc.dram_tensor("y", [TCH, P, W_OUT], f16, kind="ExternalOutput")

    # loads stream on three DMA rings concurrently, byte-balanced (sync:
    # x fp8; scalar: A groups 0-11; gpsimd: A groups 12-16 then x bf16);
    # stores alternate between the sync and gpsimd rings
    X8_SPLITS = [(0, 2), (2, 5), (5, 8), (8, 11), (11, 14)]
    A_SPLITS_SC = [(0, 3), (3, 6), (6, 9), (9, 12)]
    A_SPLITS_GP = [(12, 15), (15, 17)]
    NBANK = (W_OUT + 511) // 512            # 5 PSUM banks (last holds 2 cols)

    with tile.TileContext(nc) as tc:
        with (
            tc.tile_pool(name="xpool", bufs=1) as xpool,
            tc.tile_pool(name="apool", bufs=1) as apool,
            tc.tile_pool(name="opool", bufs=2) as opool,
            tc.tile_pool(name="idpool", bufs=1) as idpool,
            tc.tile_pool(name="psa", bufs=1, space="PSUM") as psa,
            tc.tile_pool(name="psb", bufs=2, space="PSUM") as psb,
        ):
            x8t = xpool.tile([P, N8 * _TOK_CORE], fp8, name="x8")
            x16t = xpool.tile([P, N16 * _TOK_CORE], bf16, name="x16")
            abt = apool.tile([P, tw], bf16, name="abt")
            ident = idpool.tile([P, P], bf16, name="ident")
            make_identity(nc, ident[:])

            S = _TOK_CORE
            for g0, g1 in X8_SPLITS:
                nc.sync.dma_start(x8t[:, g0 * S:g1 * S], xs8[:, g0 * S:g1 * S])
            for a0, a1 in A_SPLITS_SC:
                nc.scalar.dma_start(abt[:, offs[a0]:offs[a1]],
                                    ab[:, offs[a0]:offs[a1]])
            for a0, a1 in A_SPLITS_GP:
                o1 = tw if a1 >= NG else offs[a1]
                nc.gpsimd.dma_start(abt[:, offs[a0]:o1], ab[:, offs[a0]:o1])
            nc.gpsimd.dma_start(x16t[:], xs16[:])

            # PE warmup: >=3.4us of continuous matmuls trips the HAM clock
            # gate to 2.4 GHz while the DMAs land
            warm = psa.tile([P, P], f32, tag="warm", name="warm")
            for _ in range(36):
                nc.tensor.matmul(warm[:], ident[:], ident[:],
                                 start=True, stop=True)

            def lhsT(j, t):
                if j < N8:
                    return x8t[:, j * S + t * P:j * S + (t + 1) * P]
                return x16t[:, (j - N8) * S + t * P:(j - N8) * S + (t + 1) * P]

            # one PSUM tile per 512-col bank: drain deps are bank-granular.
            # Banks 3/4 finish at the very end of each chunk, so they get
            # double buffers; banks 0-2 drain mid-chunk and single-buffer.
            def bank_tile(t, b):
                w = min(512, W_OUT - b * 512)
                pool = psb if b >= 3 else psa
                return pool.tile([P, w], f32, tag=f"ptb{b}",
                                 name=f"pt_{t}_{b}")

            for t in range(TCH):
                pts = [bank_tile(t, b) for b in range(NBANK)]
                for i, (j, s, e) in enumerate(segs):
                    b = s // 512
                    lo2 = 2 * wins[j][0]
                    o = offs[j]
                    nc.tensor.matmul(
                        pts[b][:, s - b * 512:e - b * 512], lhsT(j, t),
                        abt[:, o + s - lo2:o + e - lo2],
                        start=(bank_first[b] == i),
                        stop=(bank_last[b] == i),
                    )
                    # keep PE duty high while chunk 0 is paced by the input
                    # DMAs so the HAM clock gate never re-throttles
                    if t == 0 and e == 2 * wins[j][1]:
                        for _ in range(2):
                            nc.tensor.matmul(warm[:], ident[:], ident[:],
                                             start=True, stop=True)
                ot = opool.tile([P, W_OUT], f16, tag="out", name=f"out_{t}")
                nc.vector.tensor_copy(ot[:, 0:512], pts[0][:])
                nc.scalar.copy(ot[:, 512:1024], pts[1][:])
                nc.sync.dma_start(y[t, :, 0:1024], ot[:, 0:1024])
                nc.vector.tensor_copy(ot[:, 1024:1536], pts[2][:])
                nc.scalar.copy(ot[:, 1536:2048], pts[3][:])
                nc.vector.tensor_copy(ot[:, 2048:W_OUT], pts[4][:])
                nc.gpsimd.dma_start(y[t, :, 1024:W_OUT], ot[:, 1024:W_OUT])

    nc.compile()
    return nc


def kernel(**inputs):
    import ml_dtypes

    x = np.ascontiguousarray(np.asarray(inputs["x"], np.float32))
    B, C, T, F = x.shape
    assert (B, C, F) == (4, _C, _F), (B, C, F)
    TS = T // _N_CORES

    A, const = _fold_matrix(
        inputs["pre_w"], inputs["pre_b"], inputs["post_w"], inputs["post_b"],
        inputs["idx"], inputs["melw"], inputs["mask"], inputs["ola_window"],
    )
    wins, offs, tw, ab, segs, bank_first, bank_last = _plan(A, const)

    key = (tw, tuple(wins))
    if key not in _PROGRAM_CACHE:
        _PROGRAM_CACHE[key] = _build_program(tw, wins, offs, segs,
                                             bank_first, bank_last)
    nc = _PROGRAM_CACHE[key]

    # host pre-shard: contraction-major x layout per core.
    # xq[ci, f, b, t] with f padded to 1088 (bias row at f=1025 == 1.0)
    NGG = _NG * _G
    xq = np.zeros((_C, NGG, B, T), np.float32)
    xq[:, :F] = x.transpose(1, 3, 0, 2)
    xq[0, F] = 1.0
    # [NG, G, C, B, T] -> partitions p = 2*f_off + ci
    xq = xq.reshape(_C, _NG, _G, B, T).transpose(1, 2, 0, 3, 4)
    x8 = (xq[:_FP8_GROUPS] * _FP8_SCALE).astype(ml_dtypes.float8_e3m4)
    x16 = xq[_FP8_GROUPS:].astype(ml_dtypes.bfloat16)

    in_maps = []
    for m in range(_N_CORES):
        sl8 = x8[:, :, :, :, m * TS:(m + 1) * TS]      # (N8, G, C, B, TS)
        sl16 = x16[:, :, :, :, m * TS:(m + 1) * TS]
        in_maps.append({
            "xs8": np.ascontiguousarray(
                sl8.reshape(_FP8_GROUPS, _P // 2 // 1, _C, _TOK_CORE)
                   .reshape(_FP8_GROUPS, _G * _C, _TOK_CORE)
                   .transpose(1, 0, 2).reshape(_P, -1)),
            "xs16": np.ascontiguousarray(
                sl16.reshape(_NG - _FP8_GROUPS, _G * _C, _TOK_CORE)
                    .transpose(1, 0, 2).reshape(_P, -1)),
            "ab": ab,
        })

    try:
        import antenv.axon_hooks  # noqa: F401
    except ImportError:
        import sys
        import types
        import antenv
        stub = types.ModuleType("antenv.axon_hooks")
        stub.get_axon_ntff_profile_hook = lambda: None
        stub.set_axon_ntff_profile_hook = lambda h: None
        sys.modules["antenv.axon_hooks"] = stub
        antenv.axon_hooks = stub

    from concourse.bass_utils import run_bass_kernel_spmd
    res = run_bass_kernel_spmd(nc, in_maps, core_ids=list(range(_N_CORES)))
    globals()["_LAST_RESULT"] = res

    out = np.empty((B, _C, T, F), np.float32)
    for m in range(_N_CORES):
        ym = res.results[m]["y"].astype(np.float32)     # (TCH, P, 2050)
        ym = ym.reshape(B, TS, F, _C)                   # tok=(b, t_local)
        out[:, :, m * TS:(m + 1) * TS, :] = ym.transpose(0, 3, 1, 2)
    return out
